# revision 15
# baseline (speedup 1.0000x reference)
"""GCN (3-layer DGL GraphConv + linear head) on 8 Trainium2 NeuronCores.

Strategy (dst-sharded message passing):
  - Nodes are sharded contiguously across 8 cores (6250/core, padded to 6272).
  - Host preprocessing (index-only work): bucket each core's in-edges by
    128-node dst window and by src half (for int16 gather indices), pad to
    128-edge chunks, and emit flat gather-index / one-hot-offset tables.
  - Device, per GCN layer:
      * dma_gather edge source rows (bf16) from the replicated node table
      * build one-hot scatter matrices S with a vector-engine is_equal
        against an iota constant
      * scatter-add via TensorE:  mT[f, d] += V[e, f]^T S[e, d] in PSUM
      * h' = m @ W  (fp32 matmul),  y = norm_dst * h' + b,  ELU,
        g_next = y * norm_src  (bf16)
      * AllGather the per-core g shard into every core's replicated table
  - Head: PE-transpose n_embed blocks, matmul with W_out, add bias.

kernel(**inputs) takes the full unsharded inputs and returns
(n_out [50000,2] f32, n_embed [50000,128] f32) like the reference.
"""

import sys

if "/opt/trn_rl_repo" not in sys.path:
    sys.path.insert(0, "/opt/trn_rl_repo")

import numpy as np
import ml_dtypes

N = 50000
E = 800000
F = 128
NC = 8
SHARD = N // NC            # 6250
WINW = 128
NW = (SHARD + WINW - 1) // WINW   # 49
SHARD_PAD = NW * WINW      # 6272
NPAD = NC * SHARD_PAD      # 50176
HALF_PAD = NPAD // 2       # 25088
GW = 4                     # windows per group
GROUPS = [(s, min(GW, NW - s)) for s in range(0, NW, GW)]  # [(0,4)...(48,1)]

BF16 = ml_dtypes.bfloat16


def _pad_row(src):
    """global node id -> row in the padded (50176) node table"""
    return (src // SHARD) * SHARD_PAD + (src % SHARD)


def preprocess(src, dst):
    """Index-only host preprocessing. Returns layout dict."""
    src = np.asarray(src).astype(np.int64)
    dst = np.asarray(dst).astype(np.int64)

    deg_out = np.bincount(src, minlength=N).astype(np.float64)
    deg_in = np.bincount(dst, minlength=N).astype(np.float64)
    norm_src = np.maximum(deg_out, 1.0) ** -0.5
    norm_dst = np.maximum(deg_in, 1.0) ** -0.5

    src_pad = _pad_row(src)
    half = (src_pad >= HALF_PAD).astype(np.int64)

    core_of = dst // SHARD
    dst_local = dst % SHARD
    win = dst_local // WINW
    off = dst_local % WINW

    # per (core, win, half) edge lists
    # order edges by (core, win, half)
    order = np.lexsort((half, win, core_of))
    e_core = core_of[order]
    e_win = win[order]
    e_half = half[order]
    e_off = off[order]
    e_srcpad = src_pad[order]

    # counts[core, win, half]
    counts = np.zeros((NC, NW, 2), np.int64)
    np.add.at(counts, (e_core, e_win, e_half), 1)
    nchunks = (counts + 127) // 128
    cpw_lo = int(nchunks[:, :, 0].max())
    cpw_hi = int(nchunks[:, :, 1].max())
    cpwt = cpw_lo + cpw_hi

    # slot tables per core:
    #   vidx[core, win, chunk(cpwt), p]  int16 gather index (into half table)
    #   soff[core, win, chunk(cpwt), p]  one-hot offset (255 = padding)
    vidx = np.zeros((NC, NW, cpwt, 128), np.int16)
    soff = np.full((NC, NW, cpwt, 128), 255, np.int64)

    # fill: for each (core, win, half) bucket, place its edges into
    # consecutive slots of its chunk range
    starts = np.zeros((NC, NW, 2), np.int64)
    # compute start index of each bucket in the sorted edge array
    flat_counts = counts.reshape(-1)
    flat_starts = np.concatenate([[0], np.cumsum(flat_counts)[:-1]])
    starts = flat_starts.reshape(NC, NW, 2)

    for c in range(NC):
        for h in range(2):
            base_chunk = 0 if h == 0 else cpw_lo
            n_cw = counts[c, :, h]
            st = starts[c, :, h]
            for w in range(NW):
                n = n_cw[w]
                if n == 0:
                    continue
                s0 = st[w]
                sl = slice(s0, s0 + n)
                idxs = e_srcpad[sl] - (HALF_PAD if h else 0)
                offs = e_off[sl]
                chunk = base_chunk + np.arange(n) // 128
                p = np.arange(n) % 128
                vidx[c, w, chunk, p] = idxs.astype(np.int16)
                soff[c, w, chunk, p] = offs

    # --- device table layouts ---
    # gather calls: per (group, half): num_idxs = gw*cpw_half*128,
    # element i = ((w_in_grp*cpw_half + chunk)*128 + p)
    # gidx[16 partitions, cols]: element i of a call at [i%16, base + i//16]
    # soffs[128, cols]: col = callchunkbase + (w_in_grp*cpw_half + chunk),
    #                   row = p  (bf16, group-major like the calls)
    call_specs = []   # (half, col0, num_idxs, chunk_col0, n_chunks, grp)
    gidx_cols = 0
    chunk_cols = 0
    for (w0, gw) in GROUPS:
        for h in range(2):
            cpw_h = cpw_lo if h == 0 else cpw_hi
            m = gw * cpw_h * 128
            call_specs.append((h, gidx_cols, m, chunk_cols, gw * cpw_h, w0, gw))
            gidx_cols += m // 16
            chunk_cols += gw * cpw_h

    gidx = np.zeros((NC, 128, gidx_cols), np.int16)
    soffs = np.full((NC, 128, chunk_cols), 255.0, np.float32)
    for c in range(NC):
        for (h, col0, m, ccol0, nch, w0, gw) in call_specs:
            cpw_h = cpw_lo if h == 0 else cpw_hi
            base_chunk = 0 if h == 0 else cpw_lo
            # [gw, cpw_h, 128]
            vi = vidx[c, w0:w0 + gw, base_chunk:base_chunk + cpw_h, :]
            so = soff[c, w0:w0 + gw, base_chunk:base_chunk + cpw_h, :]
            flat = vi.reshape(-1)  # element order (w_in_grp, chunk, p)
            i = np.arange(m)
            gidx[c, i % 16, col0 + i // 16] = flat
            soffs[c, :, ccol0:ccol0 + nch] = so.reshape(nch, 128).T
    # the Q7 descriptor generators read a 16-partition stripe each;
    # replicate the index plane across all 8 stripes
    for g in range(1, 8):
        gidx[:, g * 16:(g + 1) * 16, :] = gidx[:, 0:16, :]

    return dict(
        norm_src=norm_src.astype(np.float32),
        norm_dst=norm_dst.astype(np.float32),
        cpw_lo=cpw_lo, cpw_hi=cpw_hi, cpwt=cpwt,
        call_specs=call_specs,
        gidx=gidx, soffs=soffs,
        gidx_cols=gidx_cols, chunk_cols=chunk_cols,
    )


def _win_vec(vec_shard):
    """[SHARD] -> [128, NW] with [p, w] = vec[w*128+p] (pad -> 1.0)"""
    out = np.ones((128, NW), np.float32)
    v = np.ones(SHARD_PAD, np.float32)
    v[:SHARD] = vec_shard
    out[:, :] = v.reshape(NW, 128).T
    return out


def make_in_maps(inputs, prep):
    x = np.asarray(inputs["x"], np.float32)
    norm_src, norm_dst = prep["norm_src"], prep["norm_dst"]

    x_pad = np.zeros((NPAD, F), np.float32)
    for c in range(NC):
        x_pad[c * SHARD_PAD:c * SHARD_PAD + SHARD] = x[c * SHARD:(c + 1) * SHARD]
    x_bf = x_pad.astype(BF16)

    ns_full = np.ones((128, NPAD // 128), np.float32)
    nsp = np.ones(NPAD, np.float32)
    for c in range(NC):
        nsp[c * SHARD_PAD:c * SHARD_PAD + SHARD] = \
            norm_src[c * SHARD:(c + 1) * SHARD]
    ns_full[:, :] = nsp.reshape(-1, 128).T

    cpw_max = max(prep["cpw_lo"], prep["cpw_hi"])
    iota_w = np.tile(np.arange(128, dtype=np.float32), GW * cpw_max)
    iota_w = np.broadcast_to(iota_w, (128, GW * cpw_max * 128)).copy()

    def bwide(b, rep, w):
        return np.broadcast_to(np.tile(np.asarray(b, np.float32), rep),
                               (128, rep * w)).copy()

    common = dict(
        x_pad=x_bf,
        ns_full=ns_full,
        iota_w=iota_w.astype(BF16),
        w1=np.asarray(inputs["W1"], np.float32),
        w2=np.asarray(inputs["W2"], np.float32),
        w3=np.asarray(inputs["W3"], np.float32),
        wout=np.asarray(inputs["W_out"], np.float32),
        b1w=bwide(inputs["b1"], GW, F),
        b2w=bwide(inputs["b2"], GW, F),
        b3w=bwide(inputs["b3"], GW, F),
        boutw=bwide(inputs["b_out"], GW, 2),
    )
    in_maps = []
    for c in range(NC):
        m = dict(common)
        m["gidx"] = prep["gidx"][c]
        m["soffs"] = prep["soffs"][c].astype(BF16)
        m["normdst"] = _win_vec(norm_dst[c * SHARD:(c + 1) * SHARD])
        m["normsrc_sh"] = _win_vec(norm_src[c * SHARD:(c + 1) * SHARD])
        in_maps.append(m)
    return in_maps


# ---------------------------------------------------------------------------
# device program
# ---------------------------------------------------------------------------

def build_program(prep, use_collective=True):
    import concourse.bass as bass
    import concourse.bacc as bacc
    import concourse.tile as tile
    import concourse.mybir as mybir
    from concourse.masks import make_identity

    f32 = mybir.dt.float32
    bf16 = mybir.dt.bfloat16
    i16 = mybir.dt.int16
    AOP = mybir.AluOpType

    cpw_lo, cpw_hi = prep["cpw_lo"], prep["cpw_hi"]
    cpw_max = max(cpw_lo, cpw_hi)
    call_specs = prep["call_specs"]

    nc = bacc.Bacc("TRN2", target_bir_lowering=False, debug=False,
                   num_devices=NC)

    # --- dram tensors ---
    gidx_d = nc.dram_tensor("gidx", [128, prep["gidx_cols"]], i16,
                            kind="ExternalInput")
    soffs_d = nc.dram_tensor("soffs", [128, prep["chunk_cols"]], bf16,
                             kind="ExternalInput")
    iota_d = nc.dram_tensor("iota_w", [128, GW * cpw_max * 128], bf16,
                            kind="ExternalInput")
    ndst_d = nc.dram_tensor("normdst", [128, NW], f32, kind="ExternalInput")
    nsrc_d = nc.dram_tensor("normsrc_sh", [128, NW], f32, kind="ExternalInput")
    nsf_d = nc.dram_tensor("ns_full", [128, NPAD // 128], f32,
                           kind="ExternalInput")
    x_d = nc.dram_tensor("x_pad", [NPAD, F], bf16, kind="ExternalInput")
    w_d = [nc.dram_tensor(n, [128, 128], f32, kind="ExternalInput")
           for n in ("w1", "w2", "w3")]
    wout_d = nc.dram_tensor("wout", [128, 2], f32, kind="ExternalInput")
    bw_d = [nc.dram_tensor(n, [128, GW * F], f32, kind="ExternalInput")
            for n in ("b1w", "b2w", "b3w")]
    boutw_d = nc.dram_tensor("boutw", [128, GW * 2], f32, kind="ExternalInput")

    ne_out_d = nc.dram_tensor("ne_out", [SHARD_PAD, F], f32,
                              kind="ExternalOutput")
    no_out_d = nc.dram_tensor("no_out", [SHARD_PAD, 2], f32,
                              kind="ExternalOutput")

    g_bounce_d = nc.dram_tensor("g_bounce", [SHARD_PAD, F], bf16,
                                kind="Internal")
    g_full_d = nc.dram_tensor("g_full", [NPAD, F], bf16, kind="Internal")

    g_full_r = g_full_d.ap().rearrange("(v p) f -> p v f", p=128)
    g_bounce_r = g_bounce_d.ap().rearrange("(w p) f -> p w f", p=128)
    x_r = x_d.ap().rearrange("(v p) f -> p v f", p=128)
    ne_out_r = ne_out_d.ap().rearrange("(w p) f -> p w f", p=128)
    no_out_r = no_out_d.ap().rearrange("(w p) o -> p w o", p=128)

    with tile.TileContext(nc) as tc:
        with tc.tile_pool(name="const", bufs=1) as cpool, \
             tc.tile_pool(name="strip", bufs=1) as spool, \
             tc.tile_pool(name="work", bufs=2) as wpool, \
             tc.tile_pool(name="gath", bufs=2) as gpool, \
             tc.tile_pool(name="psA", bufs=2, space="PSUM") as psA, \
             tc.tile_pool(name="psB", bufs=2, space="PSUM") as psB, \
             tc.tile_pool(name="psC", bufs=2, space="PSUM") as psC, \
             tc.tile_pool(name="psD", bufs=2, space="PSUM") as psD:

            # ---- load constants ----
            def cload(dram, shape, dtype, nm):
                t = cpool.tile(shape, dtype, name=nm, tag=nm)
                nc.sync.dma_start(out=t[:], in_=dram.ap())
                return t

            gidx_t = cload(gidx_d, [128, prep["gidx_cols"]], i16, "c_gidx")
            soffs_t = cload(soffs_d, [128, prep["chunk_cols"]], bf16,
                            "c_soffs")
            iota_t = cload(iota_d, [128, GW * cpw_max * 128], bf16, "c_iota")
            ndst_t = cload(ndst_d, [128, NW], f32, "c_ndst")
            nsrc_t = cload(nsrc_d, [128, NW], f32, "c_nsrc")
            nsf_t = cload(nsf_d, [128, NPAD // 128], f32, "c_nsf")
            w_t = [cload(d, [128, 128], f32, f"c_w{i}")
                   for i, d in enumerate(w_d)]
            wout_t = cload(wout_d, [128, 2], f32, "c_wout")
            bw_t = [cload(d, [128, GW * F], f32, f"c_bw{i}")
                    for i, d in enumerate(bw_d)]
            boutw_t = cload(boutw_d, [128, GW * 2], f32, "c_boutw")
            ident_t = cpool.tile([128, 128], f32)
            make_identity(nc, ident_t[:])

            mT_sb = spool.tile([128, SHARD_PAD], f32, tag="mt")
            ne_sb = spool.tile([128, SHARD_PAD], f32, tag="ne")

            # ---- pre-phase: g1 = x * norm_src, written locally ----
            XB = 8
            for v0 in range(0, NPAD // 128, XB):
                xt = wpool.tile([128, XB, 128], bf16, tag="xt", bufs=3)
                gt = wpool.tile([128, XB, 128], bf16, tag="gt", bufs=3)
                nc.sync.dma_start(out=xt[:], in_=x_r[:, v0:v0 + XB, :])
                nc.vector.tensor_tensor(
                    out=gt[:], in0=xt[:],
                    in1=nsf_t[:, v0:v0 + XB].to_broadcast([128, XB, 128]),
                    op=AOP.mult)
                nc.sync.dma_start(out=g_full_r[:, v0:v0 + XB, :], in_=gt[:])

            # ---- layers ----
            for layer in range(3):
                for gi, (w0, gw) in enumerate(GROUPS):
                    mt_ps = psA.tile([128, GW * 128], f32, tag="mt_ps")
                    # gather + one-hot + scatter matmuls, lo then hi
                    for h in range(2):
                        spec = call_specs[2 * gi + h]
                        (_h, col0, m, ccol0, nch, _w0, _gw) = spec
                        cpw_h = cpw_lo if h == 0 else cpw_hi
                        table = (g_full_d.ap()[0:HALF_PAD, :] if h == 0
                                 else g_full_d.ap()[HALF_PAD:NPAD, :])
                        V = gpool.tile([128, m // 128, 128], bf16,
                                       tag=f"v{h}")
                        nc.gpsimd.dma_gather(
                            V[:], table, gidx_t[:, col0:col0 + m // 16],
                            m, m, 128, single_packet=False)
                        S = gpool.tile([128, nch, 128], bf16, tag=f"s{h}")
                        iota3 = iota_t[:].rearrange(
                            "p (c f) -> p c f", f=128)[:, :nch, :]
                        nc.vector.tensor_tensor(
                            out=S[:],
                            in0=soffs_t[:, ccol0:ccol0 + nch]
                                .to_broadcast([128, nch, 128]),
                            in1=iota3, op=AOP.is_equal)
                        for k in range(nch):
                            wi = k // cpw_h
                            first = (h == 0 and k == 0)
                            last = (h == 1 and k == nch - 1)
                            nc.tensor.matmul(
                                out=mt_ps[:, wi * 128:(wi + 1) * 128],
                                lhsT=V[:, k, :], rhs=S[:, k, :],
                                start=first, stop=last)
                    # evacuate mT
                    nc.vector.tensor_copy(
                        out=mT_sb[:, w0 * 128:(w0 + gw) * 128],
                        in_=mt_ps[:, :gw * 128])
                    # W matmul
                    h_ps = psB.tile([128, GW * 128], f32, tag="h_ps")
                    for wi in range(gw):
                        nc.tensor.matmul(
                            out=h_ps[:, wi * 128:(wi + 1) * 128],
                            lhsT=mT_sb[:, (w0 + wi) * 128:(w0 + wi + 1) * 128],
                            rhs=w_t[layer][:], start=True, stop=True)
                    # post: y = h*norm_dst + b
                    nd3 = ndst_t[:, w0:w0 + gw].to_broadcast([128, gw, 128])
                    if layer < 2:
                        y = wpool.tile([128, GW, 128], f32, tag="y")
                        nc.vector.tensor_tensor(
                            out=y[:, :gw, :],
                            in0=h_ps[:].rearrange(
                                "p (a b) -> p a b", b=128)[:, :gw, :],
                            in1=nd3, op=AOP.mult)
                        nc.vector.tensor_tensor(
                            out=y[:, :gw, :], in0=y[:, :gw, :],
                            in1=bw_t[layer][:].rearrange(
                                "p (a b) -> p a b", b=128)[:, :gw, :],
                            op=AOP.add)
                        # elu: max(y,0)-1 + exp(min(y,0)), then *norm_src
                        ng = wpool.tile([128, GW, 128], f32, tag="ng")
                        nc.vector.tensor_scalar_min(
                            out=ng[:, :gw, :], in0=y[:, :gw, :], scalar1=0.0)
                        nc.scalar.activation(
                            out=ng[:, :gw, :], in_=ng[:, :gw, :],
                            func=mybir.ActivationFunctionType.Exp)
                        nc.vector.tensor_scalar(
                            out=y[:, :gw, :], in0=y[:, :gw, :],
                            scalar1=0.0, scalar2=1.0,
                            op0=AOP.max, op1=AOP.subtract)
                        nc.vector.tensor_tensor(
                            out=y[:, :gw, :], in0=y[:, :gw, :],
                            in1=ng[:, :gw, :], op=AOP.add)
                        gbf = wpool.tile([128, GW, 128], bf16, tag="gbf")
                        ns3 = nsrc_t[:, w0:w0 + gw].to_broadcast(
                            [128, gw, 128])
                        nc.vector.tensor_tensor(
                            out=gbf[:, :gw, :], in0=y[:, :gw, :],
                            in1=ns3, op=AOP.mult)
                        nc.sync.dma_start(out=g_bounce_r[:, w0:w0 + gw, :],
                                          in_=gbf[:, :gw, :])
                    else:
                        ne3 = ne_sb[:].rearrange(
                            "p (a b) -> p a b", b=128)[:, w0:w0 + gw, :]
                        nc.vector.tensor_tensor(
                            out=ne3,
                            in0=h_ps[:].rearrange(
                                "p (a b) -> p a b", b=128)[:, :gw, :],
                            in1=nd3, op=AOP.mult)
                        nc.vector.tensor_tensor(
                            out=ne3, in0=ne3,
                            in1=bw_t[layer][:].rearrange(
                                "p (a b) -> p a b", b=128)[:, :gw, :],
                            op=AOP.add)
                        # head: transpose + W_out
                        tr_ps = psC.tile([128, GW * 128], f32, tag="tr_ps")
                        for wi in range(gw):
                            nc.tensor.transpose(
                                out=tr_ps[:, wi * 128:(wi + 1) * 128],
                                in_=ne_sb[:, (w0 + wi) * 128:
                                          (w0 + wi + 1) * 128],
                                identity=ident_t[:])
                        neT = wpool.tile([128, GW * 128], f32, tag="neT")
                        nc.vector.tensor_copy(out=neT[:, :gw * 128],
                                              in_=tr_ps[:, :gw * 128])
                        no_ps = psD.tile([128, GW * 2], f32, tag="no_ps")
                        for wi in range(gw):
                            nc.tensor.matmul(
                                out=no_ps[:, wi * 2:(wi + 1) * 2],
                                lhsT=neT[:, wi * 128:(wi + 1) * 128],
                                rhs=wout_t[:], start=True, stop=True)
                        no_sb = wpool.tile([128, GW * 2], f32, tag="no_sb")
                        nc.vector.tensor_tensor(
                            out=no_sb[:, :gw * 2], in0=no_ps[:, :gw * 2],
                            in1=boutw_t[:, :gw * 2], op=AOP.add)
                        nc.sync.dma_start(
                            out=no_out_r[:, w0:w0 + gw, :],
                            in_=no_sb[:].rearrange(
                                "p (a b) -> p a b", b=2)[:, :gw, :])
                if layer < 2:
                    if use_collective:
                        nc.gpsimd.collective_compute(
                            "AllGather", mybir.AluOpType.bypass,
                            replica_groups=[list(range(NC))],
                            ins=[g_bounce_d.ap()], outs=[g_full_d.ap()])
                    else:
                        # debug: local copy (numerically wrong on 7/8 rows)
                        for c in range(NC):
                            nc.sync.dma_start(
                                out=g_full_d.ap()[c * SHARD_PAD:
                                                  (c + 1) * SHARD_PAD, :],
                                in_=g_bounce_d.ap())

            # final n_embed output
            nc.sync.dma_start(
                out=ne_out_r[:, :, :],
                in_=ne_sb[:].rearrange("p (a b) -> p a b", b=128))

    nc.compile()
    return nc


_PROG_CACHE = {}


def kernel(**inputs):
    from concourse import bass_utils

    src = np.asarray(inputs["src"])
    dst = np.asarray(inputs["dst"])
    key = (src.tobytes(), dst.tobytes())
    kh = hash(key)
    if kh in _PROG_CACHE:
        prep, nc = _PROG_CACHE[kh]
    else:
        prep = preprocess(src, dst)
        nc = build_program(prep)
        _PROG_CACHE[kh] = (prep, nc)

    in_maps = make_in_maps(inputs, prep)
    res = bass_utils.run_bass_kernel_spmd(nc, in_maps,
                                          core_ids=list(range(NC)))
    n_embed = np.concatenate(
        [res.results[c]["ne_out"][:SHARD] for c in range(NC)], axis=0)
    n_out = np.concatenate(
        [res.results[c]["no_out"][:SHARD] for c in range(NC)], axis=0)
    return (n_out.astype(np.float32), n_embed.astype(np.float32))


# ---------------------------------------------------------------------------
# numpy emulation of the device program (for layout validation)
# ---------------------------------------------------------------------------

def emulate(inputs, prep, in_maps):
    def elu(x):
        return np.maximum(x, 0.0) + np.exp(np.minimum(x, 0.0)) - 1.0

    cpw_lo, cpw_hi = prep["cpw_lo"], prep["cpw_hi"]
    n_embed_full = np.zeros((NC, SHARD_PAD, F), np.float32)
    n_out_full = np.zeros((NC, SHARD_PAD, 2), np.float32)

    # pre-phase: g1 = x_pad * ns  (bf16), identical on every core
    x_bf = in_maps[0]["x_pad"].astype(np.float32)
    ns_full = in_maps[0]["ns_full"]
    nsv = ns_full.T.reshape(-1)  # [NPAD] row order v*128+p
    g_full = (x_bf * nsv[:, None]).astype(BF16)

    Ws = [in_maps[0]["w1"], in_maps[0]["w2"], in_maps[0]["w3"]]
    bs = [np.asarray(inputs["b1"]), np.asarray(inputs["b2"]),
          np.asarray(inputs["b3"])]

    for layer in range(3):
        g_next = np.zeros((NPAD, F), BF16)
        for c in range(NC):
            gidx = in_maps[c]["gidx"]
            soffs = in_maps[c]["soffs"].astype(np.float32)
            normdst = in_maps[c]["normdst"]
            normsrc = in_maps[c]["normsrc_sh"]
            mT = np.zeros((F, SHARD_PAD), np.float32)
            for (h, col0, m, ccol0, nch, w0, gw) in prep["call_specs"]:
                cpw_h = cpw_lo if h == 0 else cpw_hi
                table = g_full[:HALF_PAD] if h == 0 else g_full[HALF_PAD:]
                i = np.arange(m)
                idx = gidx[i % 16, col0 + i // 16].astype(np.int64)
                V = table[idx].astype(np.float32)      # [m, F] edge-major
                offs = soffs[:, ccol0:ccol0 + nch]     # [128, nch]
                for k in range(nch):
                    w = w0 + k // cpw_h
                    Vc = V[k * 128:(k + 1) * 128]      # [128e, F]
                    S = (offs[:, k][:, None] ==
                         np.arange(128)[None, :]).astype(np.float32)
                    mT[:, w * 128:(w + 1) * 128] += Vc.T @ S
            # W matmul + norm + bias
            hP = mT.T @ Ws[layer]                      # [SHARD_PAD, F]
            ndv = normdst.T.reshape(-1)
            y = hP * ndv[:, None] + bs[layer][None, :]
            if layer < 2:
                nsv_sh = normsrc.T.reshape(-1)
                g_sh = (elu(y) * nsv_sh[:, None]).astype(BF16)
                g_next[c * SHARD_PAD:(c + 1) * SHARD_PAD] = g_sh
            else:
                n_embed_full[c] = y
                wout = in_maps[c]["wout"]
                bout = np.asarray(inputs["b_out"])
                n_out_full[c] = y @ wout + bout[None, :]
        if layer < 2:
            g_full = g_next

    n_embed = np.concatenate([n_embed_full[c][:SHARD] for c in range(NC)])
    n_out = np.concatenate([n_out_full[c][:SHARD] for c in range(NC)])
    return n_out, n_embed


if __name__ == "__main__":
    # layout self-test vs reference
    sys.path.insert(0, "/root/problem")
    import reference

    inputs = {k: np.asarray(v) for k, v in reference.setup_inputs().items()}
    expected = reference.reference(**{k: v for k, v in inputs.items()})
    exp_out, exp_embed = np.asarray(expected[0]), np.asarray(expected[1])

    prep = preprocess(inputs["src"], inputs["dst"])
    print(f"cpw_lo={prep['cpw_lo']} cpw_hi={prep['cpw_hi']}")
    in_maps = make_in_maps(inputs, prep)
    n_out, n_embed = emulate(inputs, prep, in_maps)

    for name, got, exp in [("n_out", n_out, exp_out),
                           ("n_embed", n_embed, exp_embed)]:
        rel = np.abs(got - exp).max() / np.abs(exp).max()
        print(f"{name}: max rel err {rel:.3e}")
